# revision 16
# baseline (speedup 1.0000x reference)
"""Multi-head self-attention with RoPE on 8 Trainium2 NeuronCores.

Problem: x[2,2048,1024], Wq/Wk/Wv/Wo[1024,1024], 16 heads, d_k=64,
interleaved-pair RoPE, causal softmax, output projection.

Sharding: head-parallel tensor parallel. Core c owns heads (2c, 2c+1)
== feature rows [128c, 128c+128) of the Q/K/V projections and columns
[128c, 128c+128) of Wo's contraction dim. Each core produces a partial
out.T [1024, 4096]; the host sums the 8 partials (the "all-reduce").

On-device layout (per core):
  qT/kT  [128 f, 4096 t]  features-on-partitions, RoPE'd, batch-major t
  vbuf   [128 t%128, 32*130] V in natural [token, feat] 128-token blocks,
         each block 130 cols: [64 feats head A | 1 | 64 feats head B | 1]
         (the ones column rides along the attn matmul to produce the
         softmax denominator for free)
  scoresT blocks [128 k, <=512 q] via matmul(lhsT=kT chunk, rhs=qT chunk);
         causal mask applied by a second matmul accumulating
         identity.T @ maskadd (0 / -60) onto the scores in PSUM;
         exp on ACT (f32 psum -> bf16); attnT [65, 512] accumulated
         over k chunks; row 64 = denominator. Diagonal blocks are
         column-trimmed to their valid range.
  Normalize: denominators batched per (b,h) -> one DVE reciprocal,
         GPSIMD partition_broadcast, DVE multiply.
  O-proj: out.T[o,t] partial = wo_c.T @ attnT -> SBUF -> DRAM (f32).

RoPE trick: rows of Wq/Wk are permuted host-side so each head's features
are [even pairs | odd pairs]; scores are invariant to a per-head feature
permutation applied to both Q and K. Rotation is then 32-row block
copies + elementwise mul/add against host-precomputed cos/sin tables
(sin pre-signed). The 1/sqrt(d_k) scale is folded into Wq host-side.

Matmul inputs are bf16 (1 PE cycle/row + fast weight load); all
accumulation (PSUM), softmax statistics, and the output are fp32.
"""

import sys

for _p in ("/opt/trn_rl_repo",):
    if _p not in sys.path:
        sys.path.insert(0, _p)

import numpy as np

B, S, D, H, DK = 2, 2048, 1024, 16, 64
T = B * S                     # 4096 tokens, batch-major
NCORES = 8
HPC = H // NCORES             # 2 heads per core
FPC = HPC * DK                # 128 features per core
THETA = 10000.0

VW = 2 * (DK + 1)             # 130: vbuf block width (2 heads + 2 ones cols)
NKBLK = T // 128              # 32 token blocks in vbuf
MASKVAL = -60.0               # additive causal mask (exp(-60+s) underflows)

_NC_CACHE = {}


# ----------------------------------------------------------------------
# Host-side prep
# ----------------------------------------------------------------------

def _bf16(a):
    import ml_dtypes
    return np.asarray(a, np.float32).astype(ml_dtypes.bfloat16)


def _perm64():
    # de-interleave within one head: [0,2,...,62, 1,3,...,63]
    return np.concatenate([np.arange(0, DK, 2), np.arange(1, DK, 2)])


def _rope_tables():
    i = np.arange(DK // 2, dtype=np.float64)
    freqs = THETA ** (-2.0 * i / DK)                     # [32]
    pos = np.arange(S, dtype=np.float64)
    ang = pos[None, :] * freqs[:, None]                  # [32, S]
    cos32 = np.cos(ang)
    sin32 = np.sin(ang)
    cosm = np.tile(cos32, (4, 1))                        # [128, S]
    sinm = np.tile(sin32, (4, 1))
    sinm[0:32] *= -1.0                                   # even-out rows: -sin
    sinm[64:96] *= -1.0
    return cosm, sinm


def _mask_table():
    # additive mask for a diagonal block: 0 where key i <= query j, else -60
    i = np.arange(128)[:, None]
    j = np.arange(512)[None, :]
    return np.where(i <= j, 0.0, MASKVAL)


def _host_prep(x, Wq, Wk, Wv, Wo):
    xf = np.asarray(x, np.float32).reshape(T, D)
    xT = _bf16(xf.T)                                     # [D, T]
    cosm, sinm = _rope_tables()
    p64 = _perm64()

    common = {
        "xT": xT,
        "cosm": _bf16(cosm), "sinm": _bf16(sinm),
        "maskb": _bf16(_mask_table()),
        "identd": _bf16(np.eye(128)),
        "onesd": _bf16(np.ones((128, 64))),
    }
    in_maps = []
    for c in range(NCORES):
        rows = np.concatenate([(2 * c) * DK + p64, (2 * c + 1) * DK + p64])
        m = dict(common)
        m["wq"] = _bf16((np.asarray(Wq, np.float32)[rows] / np.sqrt(DK)).T)
        m["wk"] = _bf16(np.asarray(Wk, np.float32)[rows].T)
        m["wv"] = _bf16(np.asarray(Wv, np.float32)[c * FPC:(c + 1) * FPC].T)
        m["wo"] = _bf16(np.asarray(Wo, np.float32)[:, c * FPC:(c + 1) * FPC].T)
        in_maps.append(m)
    return in_maps


# ----------------------------------------------------------------------
# Device kernel
# ----------------------------------------------------------------------

def _build_nc(debug=False):
    key = ("nc", debug)
    if key in _NC_CACHE:
        return _NC_CACHE[key]

    import concourse.mybir as mybir
    import concourse.tile as tile
    from concourse import bacc

    f32 = mybir.dt.float32
    bf16 = mybir.dt.bfloat16

    nc = bacc.Bacc("TRN2", target_bir_lowering=False, debug=False)

    xT = nc.dram_tensor("xT", [D, T], bf16, kind="ExternalInput")
    wq_d = nc.dram_tensor("wq", [D, FPC], bf16, kind="ExternalInput")
    wk_d = nc.dram_tensor("wk", [D, FPC], bf16, kind="ExternalInput")
    wv_d = nc.dram_tensor("wv", [D, FPC], bf16, kind="ExternalInput")
    wo_d = nc.dram_tensor("wo", [FPC, D], bf16, kind="ExternalInput")
    cos_d = nc.dram_tensor("cosm", [128, S], bf16, kind="ExternalInput")
    sin_d = nc.dram_tensor("sinm", [128, S], bf16, kind="ExternalInput")
    msk_d = nc.dram_tensor("maskb", [128, 512], bf16, kind="ExternalInput")
    idn_d = nc.dram_tensor("identd", [128, 128], bf16, kind="ExternalInput")
    one_d = nc.dram_tensor("onesd", [128, 64], bf16, kind="ExternalInput")
    outp = nc.dram_tensor("outp", [D, T], f32, kind="ExternalOutput")
    if debug:
        dbg = {nm: nc.dram_tensor(f"dbg_{nm}", shp, bf16, kind="ExternalOutput")
               for nm, shp in [("qT", [128, T]), ("kT", [128, T]),
                               ("vbuf", [128, NKBLK * VW]),
                               ("attnT", [128, T])]}

    Exp = mybir.ActivationFunctionType.Exp
    MUL = mybir.AluOpType.mult
    ADD = mybir.AluOpType.add

    from contextlib import ExitStack

    with tile.TileContext(nc) as tc, ExitStack() as es, \
            nc.allow_low_precision(reason="bf16 matmul inputs; fp32 accum"):
        persist = es.enter_context(tc.tile_pool(name="persist", bufs=1))
        xt_pool = es.enter_context(tc.tile_pool(name="xt", bufs=2))
        work = es.enter_context(tc.tile_pool(name="work", bufs=3))
        rope_p = es.enter_context(tc.tile_pool(name="rope", bufs=2))
        ps_a = es.enter_context(tc.tile_pool(name="ps_a", bufs=4, space="PSUM"))
        ps_s = es.enter_context(tc.tile_pool(name="ps_s", bufs=2, space="PSUM"))
        ps_at = es.enter_context(tc.tile_pool(name="ps_at", bufs=2, space="PSUM"))

        qT = persist.tile([128, T], bf16, name="qT", tag="qT")
        kT = persist.tile([128, T], bf16, name="kT", tag="kT")
        vbuf = persist.tile([128, NKBLK * VW], bf16, name="vbuf", tag="vbuf")
        attnT = persist.tile([128, T], bf16, name="attnT", tag="attnT")
        wq_sb = persist.tile([128, D], bf16, name="wq", tag="wq")
        wk_sb = persist.tile([128, D], bf16, name="wk", tag="wk")
        wv_sb = persist.tile([128, D], bf16, name="wv", tag="wv")
        wo_sb = persist.tile([128, D], bf16, name="wo", tag="wo")
        cos_sb = persist.tile([128, S], bf16, name="cos", tag="cos")
        sin_sb = persist.tile([128, S], bf16, name="sin", tag="sin")
        msk_sb = persist.tile([128, 512], bf16, name="msk", tag="msk")
        ident = persist.tile([128, 128], bf16, name="ident", tag="ident")

        # --- critical-path loads: per-chunk weights (Q first) so the
        # first projection matmul can start as soon as chunk 0 lands ---
        for w_sb, w_d in ((wq_sb, wq_d), (wk_sb, wk_d), (wv_sb, wv_d)):
            for dc in range(8):
                nc.scalar.dma_start(w_sb[:, dc * 128:(dc + 1) * 128],
                                    w_d[dc * 128:(dc + 1) * 128, :])

        def emit_deferred_consts():
            nc.scalar.dma_start(cos_sb[:], cos_d[:])
            nc.scalar.dma_start(sin_sb[:], sin_d[:])
            nc.scalar.dma_start(ident[:], idn_d[:])
            nc.scalar.dma_start(msk_sb[:], msk_d[:])
            nc.scalar.dma_start(wo_sb[:], wo_d[:])
            # ones columns in vbuf (cols 64 and 129 of each 130-wide block)
            v3 = vbuf[:].rearrange("p (blk w) -> p blk w", w=VW)
            one_src = one_d[:, 0:32].rearrange("p (a b) -> p a b", b=1)
            nc.scalar.dma_start(v3[:, :, DK:DK + 1], one_src)
            nc.scalar.dma_start(v3[:, :, 2 * DK + 1:2 * DK + 2], one_src)

        # --- projections + RoPE + V transpose, per 1024-token group ---
        def emit_proj(tp, after_dma=None):
            t0 = tp * 1024
            s0 = t0 % S
            xtile = xt_pool.tile([128, 8 * 1024], bf16, name="xt", tag="xt")
            for dc in range(8):
                nc.sync.dma_start(xtile[:, dc * 1024:(dc + 1) * 1024],
                                  xT[dc * 128:(dc + 1) * 128, t0:t0 + 1024])
            if after_dma is not None:
                after_dma()
            for proj, w_sb in (("q", wq_sb), ("k", wk_sb), ("v", wv_sb)):
                pss = [ps_a.tile([128, 512], f32, name="acc", tag="acc")
                       for _ in range(2)]
                for dc in range(8):
                    for h2 in range(2):
                        nc.tensor.matmul(
                            pss[h2][:],
                            lhsT=w_sb[:, dc * 128:(dc + 1) * 128],
                            rhs=xtile[:, dc * 1024 + h2 * 512:
                                      dc * 1024 + h2 * 512 + 512],
                            start=(dc == 0), stop=(dc == 7))
                for h2 in range(2):
                    ps = pss[h2]
                    tg = t0 + h2 * 512          # global token col
                    sg = s0 + h2 * 512          # rope table col
                    if proj in ("q", "k"):
                        dst = (qT if proj == "q" else kT)[:, tg:tg + 512]
                        raw = rope_p.tile([128, 512], bf16, name="raw",
                                          tag="raw")
                        nc.scalar.copy(raw[:], ps[:])
                        sw = rope_p.tile([128, 512], bf16, name="sw", tag="sw")
                        # 32-row block swap within each head (SBUF bf16)
                        for a, b_ in ((0, 32), (32, 0), (64, 96), (96, 64)):
                            nc.vector.tensor_copy(out=sw[a:a + 32, :],
                                                  in_=raw[b_:b_ + 32, :])
                        t1 = rope_p.tile([128, 512], bf16, name="t1", tag="t1")
                        nc.vector.tensor_tensor(
                            t1[:], sw[:], sin_sb[:, sg:sg + 512], MUL)
                        t2 = rope_p.tile([128, 512], bf16, name="t2", tag="t2")
                        nc.vector.tensor_tensor(
                            t2[:], raw[:], cos_sb[:, sg:sg + 512], MUL)
                        nc.vector.tensor_tensor(dst, t1[:], t2[:], ADD)
                    else:
                        # V: copy V.T to SBUF, PE-transpose 128x128 blocks
                        # into vbuf's [tok, feat] blocks.
                        vtmp = work.tile([128, 512], bf16, name="vtmp",
                                         tag="vtmp")
                        nc.scalar.copy(vtmp[:], ps[:])
                        for vb in range(4):
                            blk = tg // 128 + vb
                            tp_ps = ps_s.tile([128, 128], bf16, name="s",
                                              tag="s")
                            nc.tensor.transpose(
                                tp_ps[:], vtmp[:, vb * 128:(vb + 1) * 128],
                                ident[:])
                            src = tp_ps[:].rearrange("p (a b) -> p a b", b=DK)
                            dstv = vbuf[:, blk * VW:(blk + 1) * VW].rearrange(
                                "p (a b) -> p a b", b=DK + 1)[:, :, 0:DK]
                            nc.vector.tensor_copy(out=dstv, in_=src)

        # --- attention for one (batch, head, q_super of 512) ---
        def emit_attn(b, h, qs):
                q0 = b * S + qs * 512
                n_k = 4 * (qs + 1)
                at = ps_at.tile([DK + 1, 512], f32, name="at", tag="at")
                for kc in range(n_k):
                    k0 = b * S + kc * 128
                    c = kc - 4 * qs             # >=0 on diagonal chunks
                    j0 = max(c, 0) * 128        # valid local q-col start
                    nw = 512 - j0
                    sps = ps_s.tile([128, 512], f32, name="s", tag="s")
                    diag = c >= 0
                    nc.tensor.matmul(
                        sps[:, j0:512],
                        lhsT=kT[h * DK:(h + 1) * DK, k0:k0 + 128],
                        rhs=qT[h * DK:(h + 1) * DK, q0 + j0:q0 + 512],
                        start=True, stop=not diag)
                    if diag:
                        # accumulate identity.T @ mask = additive causal mask
                        nc.tensor.matmul(
                            sps[:, j0:512],
                            lhsT=ident[:],
                            rhs=msk_sb[:, 0:nw],
                            start=False, stop=True)
                    ex = work.tile([128, 512], bf16, name="ex", tag="ex")
                    nc.scalar.activation(ex[:, j0:512], sps[:, j0:512], Exp)
                    blk = b * (S // 128) + kc
                    nc.tensor.matmul(
                        at[:, j0:512],
                        lhsT=vbuf[:, blk * VW + h * (DK + 1):
                                  blk * VW + (h + 1) * (DK + 1)],
                        rhs=ex[:, j0:512],
                        start=(kc == 0), stop=(kc == n_k - 1))
                # normalize: 1/denominator (fast approx, ~2^-18 rel err),
                # broadcast across 64 partitions on GPSIMD, multiply on DVE
                den = work.tile([1, 512], f32, name="den", tag="den")
                nc.vector.tensor_copy(out=den[:], in_=at[DK:DK + 1, :])
                rc = work.tile([1, 512], f32, name="rc", tag="rc")
                nc.vector.reciprocal_approx_fast(out=rc[:], in_=den[:])
                rbs = work.tile([DK, 512], f32, name="rbs", tag="rbs", bufs=3)
                nc.gpsimd.partition_broadcast(rbs[:], rc[:])
                nc.vector.tensor_tensor(
                    attnT[h * DK:(h + 1) * DK, q0:q0 + 512],
                    at[0:DK, :], rbs[:], MUL)

        # --- output projection for one 512-token chunk ---
        def emit_oproj(tck):
            for ot in range(8):
                po = ps_a.tile([128, 512], f32, name="acc", tag="acc")
                nc.tensor.matmul(
                    po[:],
                    lhsT=wo_sb[:, ot * 128:(ot + 1) * 128],
                    rhs=attnT[:, tck * 512:(tck + 1) * 512],
                    start=True, stop=True)
                pos = work.tile([128, 512], f32, name="pos", tag="pos")
                if ot % 2 == 0:
                    nc.scalar.copy(pos[:], po[:])
                else:
                    nc.vector.tensor_copy(out=pos[:], in_=po[:])
                nc.gpsimd.dma_start(
                    outp[ot * 128:(ot + 1) * 128, tck * 512:(tck + 1) * 512],
                    pos[:])

        emit_proj(0, after_dma=emit_deferred_consts)
        emit_proj(1)
        for qs in range(4):
            for h in range(HPC):
                emit_attn(0, h, qs)
            if qs == 0:
                emit_proj(2)
            elif qs == 1:
                emit_proj(3)
            emit_oproj(qs)
        for qs in range(4):
            for h in range(HPC):
                emit_attn(1, h, qs)
            emit_oproj(4 + qs)
        if debug:
            for nm, sb in [("qT", qT), ("kT", kT), ("vbuf", vbuf),
                           ("attnT", attnT)]:
                nc.sync.dma_start(dbg[nm][:], sb[:])

    nc.compile()
    _NC_CACHE[key] = nc
    return nc


# ----------------------------------------------------------------------
# Entry point
# ----------------------------------------------------------------------

def kernel(x, Wq, Wk, Wv, Wo):
    from concourse.bass_utils import run_bass_kernel_spmd

    in_maps = _host_prep(x, Wq, Wk, Wv, Wo)
    nc = _build_nc()
    res = run_bass_kernel_spmd(nc, in_maps, list(range(NCORES)))
    acc = np.zeros((D, T), np.float64)
    for r in res.results:
        acc += r["outp"].astype(np.float64)
    return np.ascontiguousarray(acc.T.astype(np.float32)).reshape(B, S, D)


# revision 17
# speedup vs baseline: 1.1163x; 1.1163x over previous
"""Multi-head self-attention with RoPE on 8 Trainium2 NeuronCores.

Problem: x[2,2048,1024], Wq/Wk/Wv/Wo[1024,1024], 16 heads, d_k=64,
interleaved-pair RoPE, causal softmax, output projection.

Sharding: head-parallel tensor parallel. Core c owns heads (2c, 2c+1)
== feature rows [128c, 128c+128) of the Q/K/V projections and columns
[128c, 128c+128) of Wo's contraction dim. Each core produces a partial
out.T [1024, 4096]; the host sums the 8 partials (the "all-reduce").

On-device layout (per core):
  qT/kT  [128 f, 4096 t]  features-on-partitions, RoPE'd, batch-major t
  vbuf   [128 t%128, 32*130] V in natural [token, feat] 128-token blocks,
         each block 130 cols: [64 feats head A | 1 | 64 feats head B | 1]
         (the ones column rides along the attn matmul to produce the
         softmax denominator for free)
  scoresT blocks [128 k, <=512 q] via matmul(lhsT=kT chunk, rhs=qT chunk);
         causal mask applied by a second matmul accumulating
         identity.T @ maskadd (0 / -60) onto the scores in PSUM;
         exp on ACT (f32 psum -> bf16); attnT [65, 512] accumulated
         over k chunks; row 64 = denominator. Diagonal blocks are
         column-trimmed to their valid range.
  Normalize: denominators batched per (b,h) -> one DVE reciprocal,
         GPSIMD partition_broadcast, DVE multiply.
  O-proj: out.T[o,t] partial = wo_c.T @ attnT -> SBUF -> DRAM (f32).

RoPE trick: rows of Wq/Wk are permuted host-side so each head's features
are [even pairs | odd pairs]; scores are invariant to a per-head feature
permutation applied to both Q and K. Rotation is then 32-row block
copies + elementwise mul/add against host-precomputed cos/sin tables
(sin pre-signed). The 1/sqrt(d_k) scale is folded into Wq host-side.

Matmul inputs are bf16 (1 PE cycle/row + fast weight load); all
accumulation (PSUM), softmax statistics, and the output are fp32.
"""

import sys

for _p in ("/opt/trn_rl_repo",):
    if _p not in sys.path:
        sys.path.insert(0, _p)

import numpy as np

B, S, D, H, DK = 2, 2048, 1024, 16, 64
T = B * S                     # 4096 tokens, batch-major
NCORES = 8
HPC = H // NCORES             # 2 heads per core
FPC = HPC * DK                # 128 features per core
THETA = 10000.0

VW = 2 * (DK + 1)             # 130: vbuf block width (2 heads + 2 ones cols)
NKBLK = T // 128              # 32 token blocks in vbuf
MASKVAL = -60.0               # additive causal mask (exp(-60+s) underflows)

_NC_CACHE = {}


# ----------------------------------------------------------------------
# Host-side prep
# ----------------------------------------------------------------------

def _bf16(a):
    import ml_dtypes
    return np.asarray(a, np.float32).astype(ml_dtypes.bfloat16)


def _perm64():
    # de-interleave within one head: [0,2,...,62, 1,3,...,63]
    return np.concatenate([np.arange(0, DK, 2), np.arange(1, DK, 2)])


def _rope_tables():
    i = np.arange(DK // 2, dtype=np.float64)
    freqs = THETA ** (-2.0 * i / DK)                     # [32]
    pos = np.arange(S, dtype=np.float64)
    ang = pos[None, :] * freqs[:, None]                  # [32, S]
    cos32 = np.cos(ang)
    sin32 = np.sin(ang)
    cosm = np.tile(cos32, (4, 1))                        # [128, S]
    sinm = np.tile(sin32, (4, 1))
    sinm[0:32] *= -1.0                                   # even-out rows: -sin
    sinm[64:96] *= -1.0
    return cosm, sinm


def _mask_table():
    # multiplicative mask for a diagonal block: 1 where key i <= query j
    i = np.arange(128)[:, None]
    j = np.arange(512)[None, :]
    return (i <= j).astype(np.float64)


def _host_prep(x, Wq, Wk, Wv, Wo):
    xf = np.asarray(x, np.float32).reshape(T, D)
    xT = _bf16(xf.T)                                     # [D, T]
    cosm, sinm = _rope_tables()
    p64 = _perm64()

    common = {
        "xT": xT,
        "cosm": _bf16(cosm), "sinm": _bf16(sinm),
        "maskb": _bf16(_mask_table()),
        "identd": _bf16(np.eye(128)),
        "onesd": _bf16(np.ones((128, 64))),
    }
    in_maps = []
    for c in range(NCORES):
        rows = np.concatenate([(2 * c) * DK + p64, (2 * c + 1) * DK + p64])
        m = dict(common)
        m["wq"] = _bf16((np.asarray(Wq, np.float32)[rows] / np.sqrt(DK)).T)
        m["wk"] = _bf16(np.asarray(Wk, np.float32)[rows].T)
        m["wv"] = _bf16(np.asarray(Wv, np.float32)[c * FPC:(c + 1) * FPC].T)
        m["wo"] = _bf16(np.asarray(Wo, np.float32)[:, c * FPC:(c + 1) * FPC].T)
        in_maps.append(m)
    return in_maps


# ----------------------------------------------------------------------
# Device kernel
# ----------------------------------------------------------------------

def _build_nc(debug=False):
    key = ("nc", debug)
    if key in _NC_CACHE:
        return _NC_CACHE[key]

    import concourse.mybir as mybir
    import concourse.tile as tile
    from concourse import bacc

    f32 = mybir.dt.float32
    bf16 = mybir.dt.bfloat16

    nc = bacc.Bacc("TRN2", target_bir_lowering=False, debug=False)

    xT = nc.dram_tensor("xT", [D, T], bf16, kind="ExternalInput")
    wq_d = nc.dram_tensor("wq", [D, FPC], bf16, kind="ExternalInput")
    wk_d = nc.dram_tensor("wk", [D, FPC], bf16, kind="ExternalInput")
    wv_d = nc.dram_tensor("wv", [D, FPC], bf16, kind="ExternalInput")
    wo_d = nc.dram_tensor("wo", [FPC, D], bf16, kind="ExternalInput")
    cos_d = nc.dram_tensor("cosm", [128, S], bf16, kind="ExternalInput")
    sin_d = nc.dram_tensor("sinm", [128, S], bf16, kind="ExternalInput")
    msk_d = nc.dram_tensor("maskb", [128, 512], bf16, kind="ExternalInput")
    idn_d = nc.dram_tensor("identd", [128, 128], bf16, kind="ExternalInput")
    one_d = nc.dram_tensor("onesd", [128, 64], bf16, kind="ExternalInput")
    outp = nc.dram_tensor("outp", [D, T], f32, kind="ExternalOutput")
    if debug:
        dbg = {nm: nc.dram_tensor(f"dbg_{nm}", shp, bf16, kind="ExternalOutput")
               for nm, shp in [("qT", [128, T]), ("kT", [128, T]),
                               ("vbuf", [128, NKBLK * VW]),
                               ("attnT", [128, T])]}

    Exp = mybir.ActivationFunctionType.Exp
    MUL = mybir.AluOpType.mult
    ADD = mybir.AluOpType.add

    from contextlib import ExitStack

    with tile.TileContext(nc) as tc, ExitStack() as es, \
            nc.allow_low_precision(reason="bf16 matmul inputs; fp32 accum"):
        persist = es.enter_context(tc.tile_pool(name="persist", bufs=1))
        xt_pool = es.enter_context(tc.tile_pool(name="xt", bufs=2))
        work = es.enter_context(tc.tile_pool(name="work", bufs=3))
        rope_p = es.enter_context(tc.tile_pool(name="rope", bufs=2))
        ps_a = es.enter_context(tc.tile_pool(name="ps_a", bufs=3, space="PSUM"))
        ps_s = es.enter_context(tc.tile_pool(name="ps_s", bufs=3, space="PSUM"))
        ps_at = es.enter_context(tc.tile_pool(name="ps_at", bufs=2, space="PSUM"))

        qT = persist.tile([128, T], bf16, name="qT", tag="qT")
        kT = persist.tile([128, T], bf16, name="kT", tag="kT")
        vbuf = persist.tile([128, NKBLK * VW], bf16, name="vbuf", tag="vbuf")
        attnT = persist.tile([128, T], bf16, name="attnT", tag="attnT")
        wq_sb = persist.tile([128, D], bf16, name="wq", tag="wq")
        wk_sb = persist.tile([128, D], bf16, name="wk", tag="wk")
        wv_sb = persist.tile([128, D], bf16, name="wv", tag="wv")
        wo_sb = persist.tile([128, D], bf16, name="wo", tag="wo")
        cos_sb = persist.tile([128, S], bf16, name="cos", tag="cos")
        sin_sb = persist.tile([128, S], bf16, name="sin", tag="sin")
        msk_sb = persist.tile([128, 512], bf16, name="msk", tag="msk")
        ident = persist.tile([128, 128], bf16, name="ident", tag="ident")

        # --- critical-path loads: per-chunk weights (Q first) so the
        # first projection matmul can start as soon as chunk 0 lands ---
        for w_sb, w_d in ((wq_sb, wq_d), (wk_sb, wk_d), (wv_sb, wv_d)):
            for dc in range(8):
                nc.gpsimd.dma_start(w_sb[:, dc * 128:(dc + 1) * 128],
                                    w_d[dc * 128:(dc + 1) * 128, :])

        def emit_deferred_consts():
            nc.gpsimd.dma_start(cos_sb[:], cos_d[:])
            nc.gpsimd.dma_start(sin_sb[:], sin_d[:])
            nc.gpsimd.dma_start(ident[:], idn_d[:])
            nc.gpsimd.dma_start(msk_sb[:], msk_d[:])
            nc.gpsimd.dma_start(wo_sb[:], wo_d[:])
            # ones columns in vbuf (cols 64 and 129 of each 130-wide block)
            v3 = vbuf[:].rearrange("p (blk w) -> p blk w", w=VW)
            one_src = one_d[:, 0:32].rearrange("p (a b) -> p a b", b=1)
            nc.gpsimd.dma_start(v3[:, :, DK:DK + 1], one_src)
            nc.gpsimd.dma_start(v3[:, :, 2 * DK + 1:2 * DK + 2], one_src)

        # --- projections + RoPE + V transpose, per 1024-token group ---
        def emit_proj(tp, after_dma=None):
            t0 = tp * 1024
            s0 = t0 % S
            xtile = xt_pool.tile([128, 8 * 1024], bf16, name="xt", tag="xt")
            for dc in range(8):
                nc.sync.dma_start(xtile[:, dc * 1024:(dc + 1) * 1024],
                                  xT[dc * 128:(dc + 1) * 128, t0:t0 + 1024])
            if after_dma is not None:
                after_dma()
            for proj, w_sb in (("q", wq_sb), ("k", wk_sb), ("v", wv_sb)):
                pss = [ps_a.tile([128, 512], f32, name="acc", tag="acc")
                       for _ in range(2)]
                for dc in range(8):
                    for h2 in range(2):
                        nc.tensor.matmul(
                            pss[h2][:],
                            lhsT=w_sb[:, dc * 128:(dc + 1) * 128],
                            rhs=xtile[:, dc * 1024 + h2 * 512:
                                      dc * 1024 + h2 * 512 + 512],
                            start=(dc == 0), stop=(dc == 7))
                for h2 in range(2):
                    ps = pss[h2]
                    tg = t0 + h2 * 512          # global token col
                    sg = s0 + h2 * 512          # rope table col
                    if proj in ("q", "k"):
                        dst = (qT if proj == "q" else kT)[:, tg:tg + 512]
                        raw = rope_p.tile([128, 512], bf16, name="raw",
                                          tag="raw")
                        nc.scalar.copy(raw[:], ps[:])
                        sw = rope_p.tile([128, 512], bf16, name="sw", tag="sw")
                        # 32-row block swap within each head (SBUF bf16)
                        for a, b_ in ((0, 32), (32, 0), (64, 96), (96, 64)):
                            nc.vector.tensor_copy(out=sw[a:a + 32, :],
                                                  in_=raw[b_:b_ + 32, :])
                        t1 = rope_p.tile([128, 512], bf16, name="t1", tag="t1")
                        nc.vector.tensor_tensor(
                            t1[:], sw[:], sin_sb[:, sg:sg + 512], MUL)
                        t2 = rope_p.tile([128, 512], bf16, name="t2", tag="t2")
                        nc.vector.tensor_tensor(
                            t2[:], raw[:], cos_sb[:, sg:sg + 512], MUL)
                        nc.vector.tensor_tensor(dst, t1[:], t2[:], ADD)
                    else:
                        # V: copy V.T to SBUF, PE-transpose 128x128 blocks
                        # into vbuf's [tok, feat] blocks.
                        vtmp = work.tile([128, 512], bf16, name="vtmp",
                                         tag="vtmp")
                        nc.scalar.copy(vtmp[:], ps[:])
                        for vb in range(4):
                            blk = tg // 128 + vb
                            tp_ps = ps_s.tile([128, 128], bf16, name="s",
                                              tag="s")
                            nc.tensor.transpose(
                                tp_ps[:], vtmp[:, vb * 128:(vb + 1) * 128],
                                ident[:])
                            src = tp_ps[:].rearrange("p (a b) -> p a b", b=DK)
                            dstv = vbuf[:, blk * VW:(blk + 1) * VW].rearrange(
                                "p (a b) -> p a b", b=DK + 1)[:, :, 0:DK]
                            nc.vector.tensor_copy(out=dstv, in_=src)

        # --- attention for one (batch, head, q_super of 512) ---
        def emit_attn(b, h, qs):
                q0 = b * S + qs * 512
                n_k = 4 * (qs + 1)
                at = ps_at.tile([DK + 1, 512], f32, name="at", tag="at")
                for kc in range(n_k):
                    k0 = b * S + kc * 128
                    c = kc - 4 * qs             # >=0 on diagonal chunks
                    j0 = max(c, 0) * 128        # valid local q-col start
                    nw = 512 - j0
                    sps = ps_s.tile([128, 512], f32, name="s", tag="s")
                    diag = c >= 0
                    nc.tensor.matmul(
                        sps[:, j0:512],
                        lhsT=kT[h * DK:(h + 1) * DK, k0:k0 + 128],
                        rhs=qT[h * DK:(h + 1) * DK, q0 + j0:q0 + 512],
                        start=True, stop=True)
                    ex = work.tile([128, 512], bf16, name="ex", tag="ex")
                    nc.scalar.activation(ex[:, j0:512], sps[:, j0:512], Exp)
                    if diag:
                        # multiplicative 0/1 causal mask on the valid window
                        nc.vector.tensor_tensor(
                            ex[:, j0:512], ex[:, j0:512],
                            msk_sb[:, 0:nw], MUL)
                    blk = b * (S // 128) + kc
                    nc.tensor.matmul(
                        at[:, j0:512],
                        lhsT=vbuf[:, blk * VW + h * (DK + 1):
                                  blk * VW + (h + 1) * (DK + 1)],
                        rhs=ex[:, j0:512],
                        start=(kc == 0), stop=(kc == n_k - 1))
                # normalize: 1/denominator (fast approx, ~2^-18 rel err),
                # broadcast across 64 partitions on GPSIMD, multiply on DVE
                den = work.tile([1, 512], f32, name="den", tag="den")
                nc.vector.tensor_copy(out=den[:], in_=at[DK:DK + 1, :])
                rc = work.tile([1, 512], f32, name="rc", tag="rc")
                nc.vector.reciprocal_approx_fast(out=rc[:], in_=den[:])
                rbs = work.tile([DK, 512], f32, name="rbs", tag="rbs", bufs=3)
                nc.gpsimd.partition_broadcast(rbs[:], rc[:])
                nc.vector.tensor_tensor(
                    attnT[h * DK:(h + 1) * DK, q0:q0 + 512],
                    at[0:DK, :], rbs[:], MUL)

        # --- output projection for one 512-token chunk ---
        def emit_oproj(tck):
            for ot in range(8):
                po = ps_a.tile([128, 512], f32, name="acc", tag="acc")
                nc.tensor.matmul(
                    po[:],
                    lhsT=wo_sb[:, ot * 128:(ot + 1) * 128],
                    rhs=attnT[:, tck * 512:(tck + 1) * 512],
                    start=True, stop=True)
                pos = work.tile([128, 512], f32, name="pos", tag="pos")
                if ot % 2 == 0:
                    nc.scalar.copy(pos[:], po[:])
                else:
                    nc.vector.tensor_copy(out=pos[:], in_=po[:])
                nc.gpsimd.dma_start(
                    outp[ot * 128:(ot + 1) * 128, tck * 512:(tck + 1) * 512],
                    pos[:])

        emit_proj(0, after_dma=emit_deferred_consts)
        emit_proj(1)
        for h in range(HPC):
            for qs in range(4):
                emit_attn(0, h, qs)
        emit_proj(2)
        emit_proj(3)
        for tck in range(4):
            emit_oproj(tck)
        for h in range(HPC):
            for qs in range(4):
                emit_attn(1, h, qs)
        for tck in range(4, 8):
            emit_oproj(tck)
        if debug:
            for nm, sb in [("qT", qT), ("kT", kT), ("vbuf", vbuf),
                           ("attnT", attnT)]:
                nc.sync.dma_start(dbg[nm][:], sb[:])

    nc.compile()
    _NC_CACHE[key] = nc
    return nc


# ----------------------------------------------------------------------
# Entry point
# ----------------------------------------------------------------------

def kernel(x, Wq, Wk, Wv, Wo):
    from concourse.bass_utils import run_bass_kernel_spmd

    in_maps = _host_prep(x, Wq, Wk, Wv, Wo)
    nc = _build_nc()
    res = run_bass_kernel_spmd(nc, in_maps, list(range(NCORES)))
    acc = np.zeros((D, T), np.float64)
    for r in res.results:
        acc += r["outp"].astype(np.float64)
    return np.ascontiguousarray(acc.T.astype(np.float32)).reshape(B, S, D)
